# revision 1
# baseline (speedup 1.0000x reference)
"""Self-contained Trainium2 Bass kernel for nn_MultiHeadAttention_71528385347884.

Strategy: head tensor-parallel across 8 cores (2 heads/core). Per core:
  - QKV projection with x transposed (feature-major q/k, token-major v)
  - RoPE via host-side A/B weight-column packing (no cross-partition ops)
  - causal attention in [s,t] score layout, softmax without max-subtraction
    (scores are bounded ~|4.5|), denominator via all-ones matmul
  - output projection exploits the reference's scrambled
    transpose(0,2,1,3).reshape(B,T,C): each core produces disjoint output
    rows -> host gather is pure concatenation.
"""

import math
import numpy as np
import ml_dtypes

# ---- problem constants (hardcoded; kernel.py must not read spec/reference) ----
B = 2
T = 2048          # sequence length per batch
C = 2048          # model dim
Dh = 128          # head dim
N_HEAD = 16
N_CORES = 8
H_LOCAL = 2       # heads per core
ROPE_BASE = 10000.0
SCALE = 1.0 / math.sqrt(Dh)

BF16 = ml_dtypes.bfloat16


class Cfg:
    """Size parameters so the same builder runs a small CoreSim config."""

    def __init__(self, B=B, T=T, C=C):
        assert T % 512 == 0 and C % 128 == 0
        self.B = B
        self.T = T
        self.C = C
        self.NCC = C // 128        # contraction chunks for qkv matmuls
        self.BT = B * T
        self.NT = T // 512         # 512-wide t-tiles per batch
        self.GRP = C // Dh         # tokens folded per output row by the reshape
        self.TAU = T // self.GRP   # output rows per (b, h); must be 128
        assert self.TAU == 128
        self.ET = max(1, C // 512)  # 512-wide e-tiles of the output
        self.JQK = 4 * 128         # qA,qB,kA,kB feature blocks
        self.JV = H_LOCAL * 128


FULL = Cfg()


# =====================================================================
# Device program builder
# =====================================================================

def build_nc(cfg: Cfg, debug=False, repeat=1):
    import concourse.bass as bass
    import concourse.mybir as mybir
    import concourse.tile as tile
    from concourse import bacc

    f32 = mybir.dt.float32
    bf16 = mybir.dt.bfloat16
    Exp = mybir.ActivationFunctionType.Exp
    Copy = mybir.ActivationFunctionType.Copy

    nc = bacc.Bacc(None, target_bir_lowering=False, debug=debug)

    xt_d = nc.dram_tensor("xt", [128, cfg.NCC, cfg.BT], bf16, kind="ExternalInput")
    wqk_d = nc.dram_tensor("wqk", [128, cfg.NCC, cfg.JQK], bf16, kind="ExternalInput")
    wv_d = nc.dram_tensor("wv", [128, cfg.NCC, cfg.JV], bf16, kind="ExternalInput")
    wp_d = nc.dram_tensor("wp", [128, cfg.GRP, cfg.C], bf16, kind="ExternalInput")
    cc2_d = nc.dram_tensor("cc2", [128, cfg.T], bf16, kind="ExternalInput")
    spm_d = nc.dram_tensor("spm", [128, cfg.T], bf16, kind="ExternalInput")
    smp_d = nc.dram_tensor("smp", [128, cfg.T], bf16, kind="ExternalInput")
    masks_d = nc.dram_tensor("masks", [128, 4, 1024], bf16, kind="ExternalInput")
    out_d = nc.dram_tensor("out", [cfg.B, H_LOCAL, 128, cfg.C], f32,
                           kind="ExternalOutput")

    with tile.TileContext(nc) as tc:
        with tc.tile_pool(name="persist", bufs=1) as persist:
            # ---- persistent SBUF state ----
            wqk_sb = persist.tile([128, cfg.NCC, cfg.JQK], bf16, name="wqk_sb",
                                  tag="wqk_sb")
            wv_sb = persist.tile([128, cfg.NCC, cfg.JV], bf16, name="wv_sb",
                                 tag="wv_sb")
            cc2_sb = persist.tile([128, cfg.T], bf16, name="cc2_sb", tag="cc2_sb")
            spm_sb = persist.tile([128, cfg.T], bf16, name="spm_sb", tag="spm_sb")
            smp_sb = persist.tile([128, cfg.T], bf16, name="smp_sb", tag="smp_sb")
            masks_sb = persist.tile([128, 4, 1024], bf16, name="masks_sb",
                                    tag="masks_sb")
            ones_sb = persist.tile([128, 128], bf16, name="ones_sb", tag="ones_sb")

            # first weight quarter up front; the rest streams behind the
            # first x slab on the same (FIFO) sync queue.
            wstep = max(1, cfg.NCC // 4)

            def preload_w(q):
                nc.sync.dma_start(wv_sb[:, q:q + wstep, :],
                                  wv_d[:, q:q + wstep, :])
                nc.sync.dma_start(wqk_sb[:, q:q + wstep, :],
                                  wqk_d[:, q:q + wstep, :])

            preload_w(0)
            nc.vector.memset(ones_sb[:], 1.0)

            # per-(b, head-or-tile) persistent tensors; q/k are stored
            # head-contiguous ([dims 0:128 of head h] on partitions) so the
            # score matmuls contract K=128 in one shot.
            qh_sb, kh_sb = {}, {}
            v_sb, vfm_sb, attn_sb = {}, {}, {}
            for b in range(cfg.B):
                for hl in range(H_LOCAL):
                    qh_sb[(b, hl)] = persist.tile([128, cfg.T], bf16,
                                                  name=f"qh_{b}_{hl}",
                                                  tag=f"qh_{b}_{hl}")
                    kh_sb[(b, hl)] = persist.tile([128, cfg.T], bf16,
                                                  name=f"kh_{b}_{hl}",
                                                  tag=f"kh_{b}_{hl}")
                for hl in range(H_LOCAL):
                    v_sb[(b, hl)] = persist.tile(
                        [128, cfg.T // 128, 128], bf16,
                        name=f"v_{b}_{hl}", tag=f"v_{b}_{hl}")
                    vfm_sb[(b, hl)] = persist.tile(
                        [128, cfg.T], bf16,
                        name=f"vf_{b}_{hl}", tag=f"vf_{b}_{hl}")
                    attn_sb[(b, hl)] = persist.tile(
                        [128, cfg.T], bf16,
                        name=f"at_{b}_{hl}", tag=f"at_{b}_{hl}")

            for rep in range(repeat):
                # ========== Phase B: fused QKV projection + RoPE ==========
                # v is computed feature-major (N=512 moving) and flipped to
                # token-major afterwards with one transpose-DMA per head.
                with (
                    tc.tile_pool(name=f"xb_pool{rep}", bufs=4) as xb_pool,
                    tc.tile_pool(name=f"rtmp{rep}", bufs=4) as rtmp,
                    tc.tile_pool(name=f"qkps{rep}", bufs=8, space="PSUM") as qkps,
                ):
                    half = max(1, cfg.NCC // 2)
                    for b in range(cfg.B):
                        for tt in range(cfg.NT):
                            bt0 = b * cfg.T + tt * 512
                            tl = slice(tt * 512, (tt + 1) * 512)
                            pj = [qkps.tile([128, 512], f32, name=f"pj_{b}_{tt}_{j}",
                                            tag="pj") for j in range(6)]
                            xlo = xb_pool.tile([128, half, 512], bf16,
                                               name=f"xbl_{b}_{tt}", tag="xb")
                            xhi = xb_pool.tile([128, half, 512], bf16,
                                               name=f"xbh_{b}_{tt}", tag="xb")
                            qtr = max(1, half // 2)
                            nc.sync.dma_start(xlo[:, 0:qtr, :],
                                              xt_d[:, 0:qtr, bt0:bt0 + 512])
                            nc.sync.dma_start(xlo[:, qtr:half, :],
                                              xt_d[:, qtr:half, bt0:bt0 + 512])
                            nc.gpsimd.dma_start(xhi[:],
                                                xt_d[:, half:cfg.NCC,
                                                     bt0:bt0 + 512])
                            if rep == 0 and b == 0 and tt == 0:
                                for q in range(wstep, cfg.NCC, wstep):
                                    preload_w(q)
                                # tile 0's rope needs only the first 512 trig
                                # columns; keep the startup DMA window small.
                                nc.scalar.dma_start(cc2_sb[:, 0:512],
                                                    cc2_d[:, 0:512])
                                nc.scalar.dma_start(spm_sb[:, 0:512],
                                                    spm_d[:, 0:512])
                                nc.scalar.dma_start(smp_sb[:, 0:512],
                                                    smp_d[:, 0:512])
                            if rep == 0 and b == 0 and tt == min(1, cfg.NT - 1):
                                if cfg.T > 512:
                                    nc.scalar.dma_start(cc2_sb[:, 512:cfg.T],
                                                        cc2_d[:, 512:cfg.T])
                                    nc.scalar.dma_start(spm_sb[:, 512:cfg.T],
                                                        spm_d[:, 512:cfg.T])
                                    nc.scalar.dma_start(smp_sb[:, 512:cfg.T],
                                                        smp_d[:, 512:cfg.T])
                                nc.scalar.dma_start(masks_sb[:], masks_d[:])
                            for ccs in range(cfg.NCC):
                                xb = (xlo if ccs < half else xhi)[:, ccs % half, :]
                                for jc in range(4):
                                    nc.tensor.matmul(
                                        pj[jc][:],
                                        wqk_sb[:, ccs, jc * 128:(jc + 1) * 128],
                                        xb,
                                        start=(ccs == 0), stop=(ccs == cfg.NCC - 1))
                                for hl in range(H_LOCAL):
                                    nc.tensor.matmul(
                                        pj[4 + hl][:],
                                        wv_sb[:, ccs, hl * 128:(hl + 1) * 128],
                                        xb,
                                        start=(ccs == 0), stop=(ccs == cfg.NCC - 1))
                            for hl in range(H_LOCAL):
                                nc.scalar.activation(vfm_sb[(b, hl)][:, tl],
                                                     pj[4 + hl][:], Copy)
                            # rope: rotA = A*C2 + B*S+-,  rotB = B*C2 + A*S-+
                            # rotA rows 0:64 = lo(h0) -> qh0[0:64]  (in place)
                            # rotA rows 64:128 = hi(h1) -> qh1[64:128] (in place)
                            # rotB rows 0:64 = hi(h0) -> qh0[64:128] (DMA move)
                            # rotB rows 64:128 = lo(h1) -> qh1[0:64]  (DMA move)
                            for (Aps, Bps, d0, d1) in (
                                (pj[0], pj[1], qh_sb[(b, 0)], qh_sb[(b, 1)]),
                                (pj[2], pj[3], kh_sb[(b, 0)], kh_sb[(b, 1)]),
                            ):
                                # all four psum-reading muls first: frees the
                                # qkv psum banks ~1.5us earlier per pair, which
                                # is what phase C's first score tiles wait on.
                                m1 = rtmp.tile([128, 512], f32, name="m1", tag="rt")
                                m2 = rtmp.tile([128, 512], f32, name="m2", tag="rt")
                                m3 = rtmp.tile([128, 512], f32, name="m3", tag="rt")
                                m4 = rtmp.tile([128, 512], f32, name="m4", tag="rt")
                                nc.vector.tensor_mul(m1[:], Aps[:], cc2_sb[:, tl])
                                nc.vector.tensor_mul(m2[:], Bps[:], spm_sb[:, tl])
                                nc.vector.tensor_mul(m3[:], Bps[:], cc2_sb[:, tl])
                                nc.vector.tensor_mul(m4[:], Aps[:], smp_sb[:, tl])
                                nc.vector.tensor_add(d0[0:64, tl],
                                                     m1[0:64, :], m2[0:64, :])
                                nc.vector.tensor_add(d1[64:128, tl],
                                                     m1[64:128, :], m2[64:128, :])
                                rb = rtmp.tile([128, 512], bf16, name="rb",
                                               tag="rtb")
                                nc.vector.tensor_add(rb[:], m3[:], m4[:])
                                nc.gpsimd.dma_start(d0[64:128, tl], rb[0:64, :])
                                nc.gpsimd.dma_start(d1[0:64, tl], rb[64:128, :])
                        for hl in range(H_LOCAL):
                            nc.sync.dma_start_transpose(v_sb[(b, hl)][:],
                                                        vfm_sb[(b, hl)][:])

                # ================= Phase C: causal attention ==================
                with (
                    tc.tile_pool(name=f"probs_pool{rep}", bufs=6) as probs_pool,
                    tc.tile_pool(name=f"acc_pool{rep}", bufs=4) as acc_pool,
                    tc.tile_pool(name=f"rec_pool{rep}", bufs=2) as rec_pool,
                    tc.tile_pool(name=f"sps{rep}", bufs=2, space="PSUM") as sps,
                    tc.tile_pool(name=f"ops{rep}", bufs=2, space="PSUM") as ops,
                    tc.tile_pool(name=f"dps{rep}", bufs=2, space="PSUM") as dps,
                ):
                    # prefetch first proj-weight slice during attention
                    wpe_tiles = {}
                    ew = min(512, cfg.C)
                    wpe_tiles[0] = persist.tile([128, cfg.GRP, ew], bf16,
                                                name=f"wpe_0_{rep}", tag="wpe",
                                                bufs=2)
                    nc.scalar.dma_start(wpe_tiles[0][:], wp_d[:, :, 0:ew])
                    for b in range(cfg.B):
                        for tt in range(cfg.NT):
                            tl = slice(tt * 512, (tt + 1) * 512)
                            n_sc = (tt + 1) * 4
                            po = [ops.tile([128, 512], f32, name=f"po_{b}_{tt}_{h}",
                                           tag="po") for h in range(2)]
                            pd = [dps.tile([128, 512], f32,
                                           name=f"pd_{b}_{tt}_{h}", tag="pd")
                                  for h in range(2)]
                            for sc in range(n_sc):
                                sl = slice(sc * 128, (sc + 1) * 128)
                                # both heads' scores in one 2-bank psum tile:
                                # head h lives in columns [h*512, h*512+512)
                                ph = sps.tile([128, 1024], f32,
                                              name=f"ps_{b}_{tt}_{sc}", tag="ps")
                                for h in range(2):
                                    nc.tensor.matmul(
                                        ph[:, h * 512:(h + 1) * 512],
                                        kh_sb[(b, h)][:, sl],
                                        qh_sb[(b, h)][:, tl],
                                        start=True, stop=True)
                                pr = probs_pool.tile([128, 1024], bf16,
                                                     name="pr", tag="pr")
                                nc.scalar.activation(pr[:], ph[:], Exp,
                                                     scale=SCALE)
                                if sc >= tt * 4:  # diagonal block: causal mask
                                    nc.vector.tensor_mul(
                                        pr[:], pr[:],
                                        masks_sb[:, sc - tt * 4, :])
                                for h in range(2):
                                    prh = pr[:, h * 512:(h + 1) * 512]
                                    nc.tensor.matmul(
                                        po[h][:], v_sb[(b, h)][:, sc, :], prh,
                                        start=(sc == 0), stop=(sc == n_sc - 1))
                                    nc.tensor.matmul(
                                        pd[h][:], ones_sb[:], prh,
                                        start=(sc == 0), stop=(sc == n_sc - 1))
                            for h in range(2):
                                rec = rec_pool.tile([128, 512], f32, name=f"rec_{h}",
                                                    tag="rec")
                                nc.vector.reciprocal(rec[:], pd[h][:])
                                nc.vector.tensor_mul(attn_sb[(b, h)][:, tl],
                                                     po[h][:], rec[:])

                # ================= Phase D: output projection =================
                with (
                    tc.tile_pool(name=f"ostg_pool{rep}", bufs=4) as ostg_pool,
                    tc.tile_pool(name=f"pps{rep}", bufs=4, space="PSUM") as pps,
                ):
                    for et in range(cfg.ET):
                        el = slice(et * 512, (et + 1) * 512)
                        if et in wpe_tiles:
                            wpe = wpe_tiles[et]
                        else:
                            wpe = persist.tile([128, cfg.GRP, ew], bf16,
                                               name=f"wpe_{et}_{rep}", tag="wpe",
                                               bufs=2)
                            nc.scalar.dma_start(wpe[:], wp_d[:, :, el])
                        for b in range(cfg.B):
                            for hl in range(H_LOCAL):
                                pp = pps.tile([128, ew], f32,
                                              name=f"pp_{et}_{b}_{hl}", tag="pp")
                                at = attn_sb[(b, hl)]
                                for u in range(cfg.GRP):
                                    nc.tensor.matmul(pp[:], at[:, u::cfg.GRP],
                                                     wpe[:, u, :],
                                                     start=(u == 0),
                                                     stop=(u == cfg.GRP - 1))
                                stg = ostg_pool.tile([128, ew], f32,
                                                     name=f"stg_{et}_{b}_{hl}",
                                                     tag="stg")
                                nc.scalar.activation(stg[:], pp[:], Copy)
                                nc.sync.dma_start(out_d[b, hl, :, el], stg[:])

    nc.compile()
    return nc


# =====================================================================
# Host-side input prep / output gather
# =====================================================================

def _part_major(a2d, ncc):
    """[ncc*128, F] -> [128, ncc, F] with row r = chunk*128 + p."""
    F = a2d.shape[1]
    return np.ascontiguousarray(
        a2d.reshape(ncc, 128, F).transpose(1, 0, 2))


def make_trig(cfg: Cfg):
    pos = np.arange(cfg.T, dtype=np.float64)[None, :]        # [1,T]
    j = np.arange(64, dtype=np.float64)[:, None]             # [64,1]
    inv = ROPE_BASE ** (-2.0 * j / Dh)
    ang = pos * inv                                          # [64,T]
    sin = np.sin(ang).astype(np.float32)
    cos = np.cos(ang).astype(np.float32)
    cc2 = np.concatenate([cos, cos], axis=0).astype(BF16)    # [128,T]
    spm = np.concatenate([-sin, sin], axis=0).astype(BF16)
    smp = np.concatenate([sin, -sin], axis=0).astype(BF16)
    return cc2, spm, smp


def make_masks():
    p = np.arange(128)[:, None]
    jj = np.arange(512)[None, :]
    masks = np.stack([((m * 128 + p) <= jj) for m in range(4)], axis=1)
    masks = np.concatenate([masks, masks], axis=2)           # [128,4,1024]
    return masks.astype(BF16)


def make_in_maps(x, w_qkv, w_proj, cfg: Cfg = FULL, n_cores=N_CORES,
                 n_head=N_HEAD):
    x = np.asarray(x, np.float32)
    w_qkv = np.asarray(w_qkv, np.float32)
    w_proj = np.asarray(w_proj, np.float32)
    Cm = cfg.C

    xT = np.ascontiguousarray(x.reshape(cfg.BT, Cm).T)       # [C, BT]
    xt = _part_major(xT, cfg.NCC).astype(BF16)
    wp = _part_major(w_proj, cfg.GRP).astype(BF16)
    cc2, spm, smp = make_trig(cfg)
    masks = make_masks()

    wq = w_qkv[:, 0:Cm]
    wk = w_qkv[:, Cm:2 * Cm]
    wv_all = w_qkv[:, 2 * Cm:3 * Cm]

    in_maps = []
    for c in range(n_cores):
        h0, h1 = 2 * c, 2 * c + 1
        q0 = wq[:, h0 * 128:(h0 + 1) * 128]
        q1 = wq[:, h1 * 128:(h1 + 1) * 128]
        k0 = wk[:, h0 * 128:(h0 + 1) * 128]
        k1 = wk[:, h1 * 128:(h1 + 1) * 128]
        qA = np.concatenate([q0[:, 0:64], q1[:, 64:128]], axis=1)
        qB = np.concatenate([q0[:, 64:128], q1[:, 0:64]], axis=1)
        kA = np.concatenate([k0[:, 0:64], k1[:, 64:128]], axis=1)
        kB = np.concatenate([k0[:, 64:128], k1[:, 0:64]], axis=1)
        wqk = _part_major(
            np.concatenate([qA, qB, kA, kB], axis=1), cfg.NCC).astype(BF16)
        wv = _part_major(
            np.concatenate([wv_all[:, h0 * 128:(h0 + 1) * 128],
                            wv_all[:, h1 * 128:(h1 + 1) * 128]], axis=1),
            cfg.NCC).astype(BF16)
        in_maps.append(dict(xt=xt, wqk=wqk, wv=wv, wp=wp,
                            cc2=cc2, spm=spm, smp=smp, masks=masks))
    return in_maps


def gather(outs, cfg: Cfg = FULL):
    """outs: per-core [B, H_LOCAL, 128, C] -> full [B, T, C]."""
    rows = np.concatenate(
        [o.reshape(cfg.B, H_LOCAL * 128, cfg.C) for o in outs], axis=1)
    return np.ascontiguousarray(rows.reshape(cfg.B, cfg.T, cfg.C))


# =====================================================================
# Public entry point
# =====================================================================

_NC_CACHE = {}


def get_nc(debug=False):
    key = ("full", debug)
    if key not in _NC_CACHE:
        _NC_CACHE[key] = build_nc(FULL, debug=debug)
    return _NC_CACHE[key]


def kernel(x, w_qkv, w_proj):
    from concourse.bass_utils import run_bass_kernel_spmd
    nc = get_nc()
    in_maps = make_in_maps(x, w_qkv, w_proj)
    res = run_bass_kernel_spmd(nc, in_maps, list(range(N_CORES)))
    return gather([res.results[c]["out"] for c in range(N_CORES)])



# revision 7
# speedup vs baseline: 1.7090x; 1.7090x over previous
"""Self-contained Trainium2 Bass kernel for nn_MultiHeadAttention_71528385347884.

Strategy: head tensor-parallel across 8 cores (2 heads/core). Per core:
  - QKV projection j-major (each feature block accumulates over all K chunks
    consecutively) so PSUM banks free quickly and RoPE overlaps.
  - RoPE in bf16: Act copies PSUM->SBUF, DVE trig combines at 2x rate.
  - causal attention in [s,t] layout; diagonal blocks compute only the valid
    t-range (partial moving dim). Softmax denominator accumulated on DVE in
    bf16 (two accumulators), collapsed across partitions with one replicated
    ones-matmul per tile; normalization multiply on the Pool engine.
  - output projection exploits the reference's scrambled
    transpose(0,2,1,3).reshape: each core produces disjoint output rows.
  - issue order is software-pipelined: projection matmuls interleave into the
    Act-bound attention windows and across the repeat boundary so the PE
    stream stays dense. PSUM: 4 banks persistent PV accumulators (parity
    alternated), 3-bank ring for transient tiles, 1 bank for projection.
"""

import math
import numpy as np
import ml_dtypes

# ---- problem constants (hardcoded; kernel.py must not read spec/reference) ----
B = 2
T = 2048          # sequence length per batch
C = 2048          # model dim
Dh = 128          # head dim
N_HEAD = 16
N_CORES = 8
H_LOCAL = 2       # heads per core
ROPE_BASE = 10000.0
SCALE = 1.0 / math.sqrt(Dh)

BF16 = ml_dtypes.bfloat16


class Cfg:
    def __init__(self, B=B, T=T, C=C):
        assert T % 512 == 0 and C % 128 == 0
        self.B = B
        self.T = T
        self.C = C
        self.NCC = C // 128        # contraction chunks for qkv matmuls
        self.BT = B * T
        self.NT = T // 512         # 512-wide t-tiles per batch
        self.GRP = C // Dh         # tokens folded per output row by the reshape
        self.TAU = T // self.GRP   # output rows per (b, h); must be 128
        assert self.TAU == 128
        self.ET = max(1, C // 512)  # 512-wide e-tiles of the output
        self.JQK = 4 * 128         # qA,qB,kA,kB feature blocks
        self.JV = H_LOCAL * 128


FULL = Cfg()


# =====================================================================
# Device program builder
# =====================================================================

def build_nc(cfg: Cfg, debug=False, repeat=1):
    import concourse.mybir as mybir
    import concourse.tile as tile
    from concourse import bacc

    f32 = mybir.dt.float32
    bf16 = mybir.dt.bfloat16
    Exp = mybir.ActivationFunctionType.Exp
    Copy = mybir.ActivationFunctionType.Copy

    nc = bacc.Bacc(None, target_bir_lowering=False, debug=debug)

    xt_d = nc.dram_tensor("xt", [128, cfg.NCC, cfg.BT], bf16, kind="ExternalInput")
    wqk_d = nc.dram_tensor("wqk", [128, cfg.NCC, cfg.JQK], bf16, kind="ExternalInput")
    wv_d = nc.dram_tensor("wv", [128, cfg.NCC, cfg.JV], bf16, kind="ExternalInput")
    wp_d = nc.dram_tensor("wp", [128, cfg.GRP, cfg.C], bf16, kind="ExternalInput")
    cc2_d = nc.dram_tensor("cc2", [128, cfg.T], bf16, kind="ExternalInput")
    smp_d = nc.dram_tensor("smp", [128, cfg.T], bf16, kind="ExternalInput")
    tri_d = nc.dram_tensor("tri", [128, 128], bf16, kind="ExternalInput")
    out_d = nc.dram_tensor("out", [cfg.B, H_LOCAL, 128, cfg.C], f32,
                           kind="ExternalOutput")

    with tile.TileContext(nc) as tc:
        with (
            tc.tile_pool(name="persist", bufs=1) as persist,
            tc.tile_pool(name="pops", bufs=1, space="PSUM") as pops,
            tc.tile_pool(name="work", bufs=2) as work,
            tc.tile_pool(name="ps", bufs=3, space="PSUM") as ps,
        ):
            # ---- persistent SBUF state ----
            wqk_sb = persist.tile([128, cfg.NCC, cfg.JQK], bf16, name="wqk_sb",
                                  tag="wqk_sb")
            wv_sb = persist.tile([128, cfg.NCC, cfg.JV], bf16, name="wv_sb",
                                 tag="wv_sb")
            cc2_sb = persist.tile([128, cfg.T], bf16, name="cc2_sb", tag="cc2_sb")
            smp_sb = persist.tile([128, cfg.T], bf16, name="smp_sb", tag="smp_sb")
            tri_sb = persist.tile([128, 128], bf16, name="tri_sb", tag="tri_sb")
            ones_sb = persist.tile([128, 128], bf16, name="ones_sb", tag="ones_sb")

            qh_sb, kh_sb, v_sb, attn_sb = {}, {}, {}, {}
            for b in range(cfg.B):
                for hl in range(H_LOCAL):
                    qh_sb[(b, hl)] = persist.tile([128, cfg.T], bf16,
                                                  name=f"qh_{b}_{hl}",
                                                  tag=f"qh_{b}_{hl}")
                    kh_sb[(b, hl)] = persist.tile([128, cfg.T], bf16,
                                                  name=f"kh_{b}_{hl}",
                                                  tag=f"kh_{b}_{hl}")
                    v_sb[(b, hl)] = persist.tile([128, cfg.T // 128, 128], bf16,
                                                 name=f"v_{b}_{hl}",
                                                 tag=f"v_{b}_{hl}")
                    attn_sb[(b, hl)] = persist.tile([128, cfg.T], bf16,
                                                    name=f"at_{b}_{hl}",
                                                    tag=f"at_{b}_{hl}")

            # persistent PSUM: PV accumulators, parity-alternated across tiles
            po_fix = {}
            for par in range(2):
                for h in range(H_LOCAL):
                    po_fix[(par, h)] = pops.tile([128, 512], f32,
                                                 name=f"pofix_{par}_{h}",
                                                 tag=f"pofix_{par}_{h}")

            # startup loads
            wstep = max(1, cfg.NCC // 4)
            nc.sync.dma_start(wv_sb[:, 0:wstep, :], wv_d[:, 0:wstep, :])
            nc.sync.dma_start(wqk_sb[:, 0:wstep, :], wqk_d[:, 0:wstep, :])
            nc.scalar.dma_start(cc2_sb[:, 0:512], cc2_d[:, 0:512])
            nc.scalar.dma_start(smp_sb[:, 0:512], smp_d[:, 0:512])
            nc.scalar.dma_start(tri_sb[:], tri_d[:])
            nc.vector.memset(ones_sb[:], 1.0)

            def psum(nm):
                return ps.tile([128, 512], f32, name=nm, tag="ps")

            # ---------- x prefetch (linear over (rep, b, tt)) ----------
            xq = []
            x_sched = [(rep, b, tt) for rep in range(repeat)
                       for b in range(cfg.B) for tt in range(cfg.NT)]
            x_next = [0]

            def prefetch_x():
                if x_next[0] >= len(x_sched):
                    return
                rep, b, tt = x_sched[x_next[0]]
                x_next[0] += 1
                bt0 = b * cfg.T + tt * 512
                xb = work.tile([128, cfg.NCC, 512], bf16,
                               name=f"xb_{rep}_{b}_{tt}", tag="xb", bufs=2)
                half = cfg.NCC // 2
                nc.sync.dma_start(xb[:, 0:half, :],
                                  xt_d[:, 0:half, bt0:bt0 + 512])
                nc.gpsimd.dma_start(xb[:, half:cfg.NCC, :],
                                    xt_d[:, half:cfg.NCC, bt0:bt0 + 512])
                xq.append(xb)

            prefetch_x()
            prefetch_x()

            # ---------- deferred work queues ----------
            pd_pending = []      # (b, tt) whose denominator collapse is due
            d_pending = []       # closures emitting one projection group each
            c_state = {}         # (b, tt) -> (po dict, accS)
            uid = [0]            # name uniquifier for deferred emissions

            def queue_D(b):
                uid[0] += 1
                u0 = uid[0]
                wpe_state = {}

                def load_wpe(et):
                    w = work.tile([128, cfg.GRP, 512], bf16,
                                  name=f"wpe_{u0}_{et}", tag="wpe", bufs=2)
                    nc.scalar.dma_start(w[:], wp_d[:, :, et * 512:(et + 1) * 512])
                    wpe_state[et] = w

                load_wpe(0)

                def make(et, hl):
                    def f():
                        if hl == 0 and et + 1 < cfg.ET:
                            load_wpe(et + 1)
                        wpe = wpe_state[et]
                        pp = ps.tile([128, 512], f32, name=f"pp_{u0}_{et}_{hl}",
                                     tag="pp", bufs=1)
                        at = attn_sb[(b, hl)]
                        for u in range(cfg.GRP):
                            nc.tensor.matmul(pp[:], at[:, u::cfg.GRP],
                                             wpe[:, u, :],
                                             start=(u == 0),
                                             stop=(u == cfg.GRP - 1))
                        stg = work.tile([128, 512], f32,
                                        name=f"stg_{u0}_{et}_{hl}",
                                        tag="stg", bufs=2)
                        nc.scalar.activation(stg[:], pp[:], Copy)
                        nc.sync.dma_start(out_d[b, hl, :, et * 512:(et + 1) * 512],
                                          stg[:])
                    return f

                for et in range(cfg.ET):
                    for hl in range(H_LOCAL):
                        d_pending.append(make(et, hl))

            def emit_pd():
                if not pd_pending:
                    return
                b, tt = pd_pending.pop(0)
                po, accS = c_state.pop((b, tt))
                tl = slice(tt * 512, (tt + 1) * 512)
                uid[0] += 1
                u0 = uid[0]
                for h in range(H_LOCAL):
                    pdh = psum(f"pd_{u0}_{h}")
                    nc.tensor.matmul(pdh[:], ones_sb[:],
                                     accS[:, h * 512:(h + 1) * 512],
                                     start=True, stop=True)
                    rec = work.tile([128, 512], f32, name=f"rec_{u0}_{h}",
                                    tag="rec", bufs=2)
                    nc.vector.reciprocal(rec[:], pdh[:])
                    nc.vector.tensor_mul(attn_sb[(b, h)][:, tl], po[h][:], rec[:])
                if tt == cfg.NT - 1:
                    queue_D(b)

            def drain_D(n=1):
                for _ in range(min(n, len(d_pending))):
                    d_pending.pop(0)()

            # ---------- phase B: QKV projection + RoPE ----------
            def rope_group(Aps, Bps, d0, d1, tl, nm):
                # A/B PSUM -> bf16, then rotA = A*cos - B*smp ; rotB = B*cos + A*smp
                ac = work.tile([128, 512], bf16, name=f"ac{nm}", tag="rc", bufs=4)
                bc = work.tile([128, 512], bf16, name=f"bc{nm}", tag="rc", bufs=4)
                nc.scalar.activation(ac[:], Aps[:], Copy)
                nc.scalar.activation(bc[:], Bps[:], Copy)
                m1 = work.tile([128, 512], bf16, name=f"m1{nm}", tag="rt", bufs=6)
                m2 = work.tile([128, 512], bf16, name=f"m2{nm}", tag="rt", bufs=6)
                m3 = work.tile([128, 512], bf16, name=f"m3{nm}", tag="rt", bufs=6)
                m4 = work.tile([128, 512], bf16, name=f"m4{nm}", tag="rt", bufs=6)
                nc.vector.tensor_mul(m1[:], ac[:], cc2_sb[:, tl])
                nc.vector.tensor_mul(m2[:], bc[:], smp_sb[:, tl])
                nc.vector.tensor_mul(m3[:], bc[:], cc2_sb[:, tl])
                nc.vector.tensor_mul(m4[:], ac[:], smp_sb[:, tl])
                nc.vector.tensor_sub(d0[0:64, tl], m1[0:64, :], m2[0:64, :])
                nc.vector.tensor_sub(d1[64:128, tl], m1[64:128, :], m2[64:128, :])
                rb = work.tile([128, 512], bf16, name=f"rb{nm}", tag="rb", bufs=2)
                nc.vector.tensor_add(rb[:], m3[:], m4[:])
                nc.gpsimd.dma_start(d0[64:128, tl], rb[0:64, :])
                nc.gpsimd.dma_start(d1[0:64, tl], rb[64:128, :])

            def emit_B(rep, b):
                for tt in range(cfg.NT):
                    xb = xq.pop(0)
                    prefetch_x()
                    tl = slice(tt * 512, (tt + 1) * 512)
                    if rep == 0 and b == 0 and tt == 0:
                        # stream the rest of the startup state behind tile 0
                        for q in range(wstep, cfg.NCC, wstep):
                            nc.sync.dma_start(wv_sb[:, q:q + wstep, :],
                                              wv_d[:, q:q + wstep, :])
                            nc.sync.dma_start(wqk_sb[:, q:q + wstep, :],
                                              wqk_d[:, q:q + wstep, :])
                        nc.scalar.dma_start(cc2_sb[:, 512:cfg.T],
                                            cc2_d[:, 512:cfg.T])
                        nc.scalar.dma_start(smp_sb[:, 512:cfg.T],
                                            smp_d[:, 512:cfg.T])
                    # j order: v0, v1, qA, qB, kA, kB
                    pj = {}
                    for j in range(6):
                        p = psum(f"pj_{rep}_{b}_{tt}_{j}")
                        for ccs in range(cfg.NCC):
                            if j < 2:
                                w = wv_sb[:, ccs, j * 128:(j + 1) * 128]
                            else:
                                jc = j - 2
                                w = wqk_sb[:, ccs, jc * 128:(jc + 1) * 128]
                            nc.tensor.matmul(p[:], w, xb[:, ccs, :],
                                             start=(ccs == 0),
                                             stop=(ccs == cfg.NCC - 1))
                        pj[j] = p
                        if j == 1:
                            for hl in range(H_LOCAL):
                                vfm = work.tile([128, 512], bf16,
                                                name=f"vfm_{rep}_{b}_{tt}_{hl}",
                                                tag="vfm", bufs=3)
                                nc.scalar.activation(vfm[:], pj[hl][:], Copy)
                                nc.sync.dma_start_transpose(
                                    v_sb[(b, hl)][:, 4 * tt:4 * tt + 4, :],
                                    vfm[:])
                            if not (rep == 0 and b == 0 and tt == 0):
                                emit_pd()
                                if tt > 0:
                                    drain_D(1)
                        elif j == 3:
                            rope_group(pj[2], pj[3], qh_sb[(b, 0)], qh_sb[(b, 1)],
                                       tl, f"q{rep}_{b}_{tt}")
                            if tt > 0:
                                drain_D(1)
                        elif j == 5:
                            rope_group(pj[4], pj[5], kh_sb[(b, 0)], kh_sb[(b, 1)],
                                       tl, f"k{rep}_{b}_{tt}")
                            if tt > 0:
                                drain_D(1)

            # ---------- phase C: causal attention ----------
            def emit_C(rep, b):
                for tt in range(cfg.NT):
                    tl0 = tt * 512
                    n_sc = 4 * (tt + 1)
                    po = {h: po_fix[(tt % 2, h)] for h in range(H_LOCAL)}
                    acc0 = work.tile([128, 1024], bf16,
                                     name=f"a0_{rep}_{b}_{tt}", tag="acc0", bufs=2)
                    acc1 = None
                    if tt > 0:
                        acc1 = work.tile([128, 1024], bf16,
                                         name=f"a1_{rep}_{b}_{tt}", tag="acc1",
                                         bufs=2)
                    for sc in range(n_sc):
                        off = max(0, (sc - 4 * tt) * 128)
                        wdt = 512 - off
                        sl = slice(sc * 128, (sc + 1) * 128)
                        tls = slice(tl0 + off, tl0 + 512)
                        ph = {}
                        for h in range(H_LOCAL):
                            p = psum(f"ph_{rep}_{b}_{tt}_{sc}_{h}")
                            nc.tensor.matmul(p[:, off:512],
                                             kh_sb[(b, h)][:, sl],
                                             qh_sb[(b, h)][:, tls],
                                             start=True, stop=True)
                            ph[h] = p
                        pr = work.tile([128, 1024], bf16,
                                       name=f"pr_{rep}_{b}_{tt}_{sc}",
                                       tag="pr", bufs=4)
                        for h in range(H_LOCAL):
                            nc.scalar.activation(pr[:, h * 512 + off:(h + 1) * 512],
                                                 ph[h][:, off:512], Exp,
                                                 scale=SCALE)
                        if sc >= 4 * tt:   # diagonal: mask the triangle block
                            for h in range(H_LOCAL):
                                pos = h * 512 + off
                                nc.gpsimd.tensor_mul(pr[:, pos:pos + 128],
                                                     pr[:, pos:pos + 128],
                                                     tri_sb[:])
                        # denominator accumulation (bf16, two chains)
                        if sc == 0:
                            nc.vector.tensor_copy(acc0[:], pr[:])
                        elif sc == 1 and tt > 0:
                            nc.vector.tensor_copy(acc1[:], pr[:])
                        else:
                            tgt = acc0 if (tt == 0 or sc % 2 == 0) else acc1
                            if off == 0:
                                nc.vector.tensor_add(tgt[:], tgt[:], pr[:])
                            else:
                                for h in range(H_LOCAL):
                                    pos = h * 512 + off
                                    nc.gpsimd.tensor_add(tgt[:, pos:pos + wdt],
                                                         tgt[:, pos:pos + wdt],
                                                         pr[:, pos:pos + wdt])
                        for h in range(H_LOCAL):
                            nc.tensor.matmul(po[h][:, off:512],
                                             v_sb[(b, h)][:, sc, :],
                                             pr[:, h * 512 + off:(h + 1) * 512],
                                             start=(sc == 0),
                                             stop=(sc == n_sc - 1))
                        if sc == 1:
                            emit_pd()
                        if sc % 5 == 4:
                            drain_D(1)
                    # fold the two accumulator chains
                    if acc1 is not None:
                        accS = work.tile([128, 1024], bf16,
                                         name=f"as_{rep}_{b}_{tt}", tag="accS",
                                         bufs=2)
                        nc.vector.tensor_add(accS[:], acc0[:], acc1[:])
                    else:
                        accS = acc0
                    c_state[(b, tt)] = (po, accS)
                    pd_pending.append((b, tt))

            # ---------- main schedule ----------
            for rep in range(repeat):
                emit_B(rep, 0)
                emit_B(rep, 1)
                emit_C(rep, 0)
                emit_C(rep, 1)

            # tail: last tile's denominator + final projection phase
            emit_pd()
            drain_D(len(d_pending))

    nc.compile()
    return nc


# =====================================================================
# Host-side input prep / output gather
# =====================================================================

def _part_major(a2d, ncc):
    """[ncc*128, F] -> [128, ncc, F] with row r = chunk*128 + p."""
    F = a2d.shape[1]
    return np.ascontiguousarray(
        a2d.reshape(ncc, 128, F).transpose(1, 0, 2))


def make_trig(cfg: Cfg):
    pos = np.arange(cfg.T, dtype=np.float64)[None, :]        # [1,T]
    j = np.arange(64, dtype=np.float64)[:, None]             # [64,1]
    inv = ROPE_BASE ** (-2.0 * j / Dh)
    ang = pos * inv                                          # [64,T]
    sin = np.sin(ang).astype(np.float32)
    cos = np.cos(ang).astype(np.float32)
    cc2 = np.concatenate([cos, cos], axis=0).astype(BF16)    # [128,T]
    smp = np.concatenate([sin, -sin], axis=0).astype(BF16)
    return cc2, smp


def make_tri():
    p = np.arange(128)[:, None]
    jj = np.arange(128)[None, :]
    return (p <= jj).astype(BF16)


def make_in_maps(x, w_qkv, w_proj, cfg: Cfg = FULL, n_cores=N_CORES,
                 n_head=N_HEAD):
    x = np.asarray(x, np.float32)
    w_qkv = np.asarray(w_qkv, np.float32)
    w_proj = np.asarray(w_proj, np.float32)
    Cm = cfg.C

    xT = np.ascontiguousarray(x.reshape(cfg.BT, Cm).T)       # [C, BT]
    xt = _part_major(xT, cfg.NCC).astype(BF16)
    wp = _part_major(w_proj, cfg.GRP).astype(BF16)
    cc2, smp = make_trig(cfg)
    tri = make_tri()

    wq = w_qkv[:, 0:Cm]
    wk = w_qkv[:, Cm:2 * Cm]
    wv_all = w_qkv[:, 2 * Cm:3 * Cm]

    in_maps = []
    for c in range(n_cores):
        h0, h1 = 2 * c, 2 * c + 1
        q0 = wq[:, h0 * 128:(h0 + 1) * 128]
        q1 = wq[:, h1 * 128:(h1 + 1) * 128]
        k0 = wk[:, h0 * 128:(h0 + 1) * 128]
        k1 = wk[:, h1 * 128:(h1 + 1) * 128]
        qA = np.concatenate([q0[:, 0:64], q1[:, 64:128]], axis=1)
        qB = np.concatenate([q0[:, 64:128], q1[:, 0:64]], axis=1)
        kA = np.concatenate([k0[:, 0:64], k1[:, 64:128]], axis=1)
        kB = np.concatenate([k0[:, 64:128], k1[:, 0:64]], axis=1)
        wqk = _part_major(
            np.concatenate([qA, qB, kA, kB], axis=1), cfg.NCC).astype(BF16)
        wv = _part_major(
            np.concatenate([wv_all[:, h0 * 128:(h0 + 1) * 128],
                            wv_all[:, h1 * 128:(h1 + 1) * 128]], axis=1),
            cfg.NCC).astype(BF16)
        in_maps.append(dict(xt=xt, wqk=wqk, wv=wv, wp=wp,
                            cc2=cc2, smp=smp, tri=tri))
    return in_maps


def gather(outs, cfg: Cfg = FULL):
    """outs: per-core [B, H_LOCAL, 128, C] -> full [B, T, C]."""
    rows = np.concatenate(
        [o.reshape(cfg.B, H_LOCAL * 128, cfg.C) for o in outs], axis=1)
    return np.ascontiguousarray(rows.reshape(cfg.B, cfg.T, cfg.C))


# =====================================================================
# Public entry point
# =====================================================================

_NC_CACHE = {}


def get_nc(debug=False):
    key = ("full", debug)
    if key not in _NC_CACHE:
        _NC_CACHE[key] = build_nc(FULL, debug=debug)
    return _NC_CACHE[key]


def kernel(x, w_qkv, w_proj):
    from concourse.bass_utils import run_bass_kernel_spmd
    nc = get_nc()
    in_maps = make_in_maps(x, w_qkv, w_proj)
    res = run_bass_kernel_spmd(nc, in_maps, list(range(N_CORES)))
    return gather([res.results[c]["out"] for c in range(N_CORES)])


# revision 8
# speedup vs baseline: 3.4361x; 2.0106x over previous
"""Self-contained Trainium2 Bass kernel for nn_MultiHeadAttention_71528385347884.

Strategy: head tensor-parallel across 8 cores (2 heads/core). Per core:
  - QKV projection j-major (each feature block accumulates over all K chunks
    consecutively) so PSUM banks free quickly and RoPE overlaps.
  - RoPE in bf16: Act copies PSUM->SBUF, DVE trig combines at 2x rate.
  - causal attention in [s,t] layout; diagonal blocks compute only the valid
    t-range (partial moving dim). Softmax denominator accumulated on DVE in
    bf16 (two accumulators), collapsed across partitions with one replicated
    ones-matmul per tile; normalization multiply on the Pool engine.
  - output projection exploits the reference's scrambled
    transpose(0,2,1,3).reshape: each core produces disjoint output rows.
  - issue order is software-pipelined: projection matmuls interleave into the
    Act-bound attention windows and across the repeat boundary so the PE
    stream stays dense. PSUM: 4 banks persistent PV accumulators (parity
    alternated), 3-bank ring for transient tiles, 1 bank for projection.
"""

import math
import numpy as np
import ml_dtypes

# ---- problem constants (hardcoded; kernel.py must not read spec/reference) ----
B = 2
T = 2048          # sequence length per batch
C = 2048          # model dim
Dh = 128          # head dim
N_HEAD = 16
N_CORES = 8
H_LOCAL = 2       # heads per core
ROPE_BASE = 10000.0
SCALE = 1.0 / math.sqrt(Dh)

BF16 = ml_dtypes.bfloat16


class Cfg:
    def __init__(self, B=B, T=T, C=C):
        assert T % 512 == 0 and C % 128 == 0
        self.B = B
        self.T = T
        self.C = C
        self.NCC = C // 128        # contraction chunks for qkv matmuls
        self.BT = B * T
        self.NT = T // 512         # 512-wide t-tiles per batch
        self.GRP = C // Dh         # tokens folded per output row by the reshape
        self.TAU = T // self.GRP   # output rows per (b, h); must be 128
        assert self.TAU == 128
        self.ET = max(1, C // 512)  # 512-wide e-tiles of the output
        self.JQK = 4 * 128         # qA,qB,kA,kB feature blocks
        self.JV = H_LOCAL * 128


FULL = Cfg()


# =====================================================================
# Device program builder
# =====================================================================

def build_nc(cfg: Cfg, debug=False, repeat=1):
    import concourse.mybir as mybir
    import concourse.tile as tile
    from concourse import bacc

    f32 = mybir.dt.float32
    bf16 = mybir.dt.bfloat16
    Exp = mybir.ActivationFunctionType.Exp
    Copy = mybir.ActivationFunctionType.Copy

    nc = bacc.Bacc(None, target_bir_lowering=False, debug=debug)

    xt_d = nc.dram_tensor("xt", [128, cfg.NCC, cfg.BT], bf16, kind="ExternalInput")
    wqk_d = nc.dram_tensor("wqk", [128, cfg.NCC, cfg.JQK], bf16, kind="ExternalInput")
    wv_d = nc.dram_tensor("wv", [128, cfg.NCC, cfg.JV], bf16, kind="ExternalInput")
    wp_d = nc.dram_tensor("wp", [128, cfg.GRP, cfg.C], bf16, kind="ExternalInput")
    cc2_d = nc.dram_tensor("cc2", [128, cfg.T], bf16, kind="ExternalInput")
    smp_d = nc.dram_tensor("smp", [128, cfg.T], bf16, kind="ExternalInput")
    tri_d = nc.dram_tensor("tri", [128, 128], bf16, kind="ExternalInput")
    out_d = nc.dram_tensor("out", [cfg.B, H_LOCAL, 128, cfg.C], f32,
                           kind="ExternalOutput")

    with tile.TileContext(nc) as tc:
        with (
            tc.tile_pool(name="persist", bufs=1) as persist,
            tc.tile_pool(name="pops", bufs=1, space="PSUM") as pops,
            tc.tile_pool(name="work", bufs=2) as work,
            tc.tile_pool(name="ps", bufs=3, space="PSUM") as ps,
        ):
            # ---- persistent SBUF state ----
            wqk_sb = persist.tile([128, cfg.NCC, cfg.JQK], bf16, name="wqk_sb",
                                  tag="wqk_sb")
            wv_sb = persist.tile([128, cfg.NCC, cfg.JV], bf16, name="wv_sb",
                                 tag="wv_sb")
            cc2_sb = persist.tile([128, cfg.T], bf16, name="cc2_sb", tag="cc2_sb")
            smp_sb = persist.tile([128, cfg.T], bf16, name="smp_sb", tag="smp_sb")
            tri_sb = persist.tile([128, 128], bf16, name="tri_sb", tag="tri_sb")
            ones_sb = persist.tile([128, 128], bf16, name="ones_sb", tag="ones_sb")

            qh_sb, kh_sb, v_sb, attn_sb = {}, {}, {}, {}
            for b in range(cfg.B):
                for hl in range(H_LOCAL):
                    qh_sb[(b, hl)] = persist.tile([128, cfg.T], bf16,
                                                  name=f"qh_{b}_{hl}",
                                                  tag=f"qh_{b}_{hl}")
                    kh_sb[(b, hl)] = persist.tile([128, cfg.T], bf16,
                                                  name=f"kh_{b}_{hl}",
                                                  tag=f"kh_{b}_{hl}")
                    v_sb[(b, hl)] = persist.tile([128, cfg.T // 128, 128], bf16,
                                                 name=f"v_{b}_{hl}",
                                                 tag=f"v_{b}_{hl}")
                    attn_sb[(b, hl)] = persist.tile([128, cfg.T], bf16,
                                                    name=f"at_{b}_{hl}",
                                                    tag=f"at_{b}_{hl}")

            # persistent PSUM: PV accumulators, parity-alternated across tiles
            po_fix = {}
            for par in range(2):
                for h in range(H_LOCAL):
                    po_fix[(par, h)] = pops.tile([128, 512], f32,
                                                 name=f"pofix_{par}_{h}",
                                                 tag=f"pofix_{par}_{h}")

            # startup loads
            wstep = max(1, cfg.NCC // 4)
            nc.sync.dma_start(wv_sb[:, 0:wstep, :], wv_d[:, 0:wstep, :])
            nc.sync.dma_start(wqk_sb[:, 0:wstep, :], wqk_d[:, 0:wstep, :])
            nc.scalar.dma_start(cc2_sb[:, 0:512], cc2_d[:, 0:512])
            nc.scalar.dma_start(smp_sb[:, 0:512], smp_d[:, 0:512])
            nc.scalar.dma_start(tri_sb[:], tri_d[:])
            nc.vector.memset(ones_sb[:], 1.0)

            def psum(nm):
                return ps.tile([128, 512], f32, name=nm, tag="ps")

            # ---------- x prefetch (linear over (rep, b, tt)) ----------
            xq = []
            x_sched = [(rep, b, tt) for rep in range(repeat)
                       for b in range(cfg.B) for tt in range(cfg.NT)]
            x_next = [0]

            def prefetch_x():
                if x_next[0] >= len(x_sched):
                    return
                rep, b, tt = x_sched[x_next[0]]
                x_next[0] += 1
                bt0 = b * cfg.T + tt * 512
                xb = work.tile([128, cfg.NCC, 512], bf16,
                               name=f"xb_{rep}_{b}_{tt}", tag="xb", bufs=2)
                half = cfg.NCC // 2
                nc.sync.dma_start(xb[:, 0:half, :],
                                  xt_d[:, 0:half, bt0:bt0 + 512])
                nc.gpsimd.dma_start(xb[:, half:cfg.NCC, :],
                                    xt_d[:, half:cfg.NCC, bt0:bt0 + 512])
                xq.append(xb)

            prefetch_x()
            prefetch_x()

            # ---------- deferred work queues ----------
            pd_pending = []      # (b, tt) whose denominator collapse is due
            d_pending = []       # closures emitting one projection group each
            c_state = {}         # (b, tt) -> (po dict, accS)
            uid = [0]            # name uniquifier for deferred emissions

            def queue_D(b):
                uid[0] += 1
                u0 = uid[0]
                wpe_state = {}

                def load_wpe(et):
                    w = work.tile([128, cfg.GRP, 512], bf16,
                                  name=f"wpe_{u0}_{et}", tag="wpe", bufs=2)
                    nc.scalar.dma_start(w[:], wp_d[:, :, et * 512:(et + 1) * 512])
                    wpe_state[et] = w

                load_wpe(0)

                def make(et, hl):
                    def f():
                        if hl == 0 and et + 1 < cfg.ET:
                            load_wpe(et + 1)
                        wpe = wpe_state[et]
                        pp = ps.tile([128, 512], f32, name=f"pp_{u0}_{et}_{hl}",
                                     tag="pp", bufs=1)
                        at = attn_sb[(b, hl)]
                        for u in range(cfg.GRP):
                            nc.tensor.matmul(pp[:], at[:, u::cfg.GRP],
                                             wpe[:, u, :],
                                             start=(u == 0),
                                             stop=(u == cfg.GRP - 1))
                        stg = work.tile([128, 512], f32,
                                        name=f"stg_{u0}_{et}_{hl}",
                                        tag="stg", bufs=2)
                        nc.scalar.activation(stg[:], pp[:], Copy)
                        nc.sync.dma_start(out_d[b, hl, :, et * 512:(et + 1) * 512],
                                          stg[:])
                    return f

                for et in range(cfg.ET):
                    for hl in range(H_LOCAL):
                        d_pending.append(make(et, hl))

            def emit_pd():
                if not pd_pending:
                    return
                b, tt = pd_pending.pop(0)
                po, accS = c_state.pop((b, tt))
                tl = slice(tt * 512, (tt + 1) * 512)
                uid[0] += 1
                u0 = uid[0]
                for h in range(H_LOCAL):
                    pdh = psum(f"pd_{u0}_{h}")
                    nc.tensor.matmul(pdh[:], ones_sb[:],
                                     accS[:, h * 512:(h + 1) * 512],
                                     start=True, stop=True)
                    rec = work.tile([128, 512], f32, name=f"rec_{u0}_{h}",
                                    tag="rec", bufs=2)
                    nc.vector.reciprocal(rec[:], pdh[:])
                    nc.vector.tensor_mul(attn_sb[(b, h)][:, tl], po[h][:], rec[:])
                if tt == cfg.NT - 1:
                    queue_D(b)

            def drain_D(n=1):
                for _ in range(min(n, len(d_pending))):
                    d_pending.pop(0)()

            # ---------- phase B: QKV projection + RoPE ----------
            def rope_group(Aps, Bps, d0, d1, tl, nm):
                # A/B PSUM -> bf16, then rotA = A*cos - B*smp ; rotB = B*cos + A*smp
                ac = work.tile([128, 512], bf16, name=f"ac{nm}", tag="rc", bufs=4)
                bc = work.tile([128, 512], bf16, name=f"bc{nm}", tag="rc", bufs=4)
                nc.scalar.activation(ac[:], Aps[:], Copy)
                nc.scalar.activation(bc[:], Bps[:], Copy)
                m1 = work.tile([128, 512], bf16, name=f"m1{nm}", tag="rt", bufs=6)
                m2 = work.tile([128, 512], bf16, name=f"m2{nm}", tag="rt", bufs=6)
                m3 = work.tile([128, 512], bf16, name=f"m3{nm}", tag="rt", bufs=6)
                m4 = work.tile([128, 512], bf16, name=f"m4{nm}", tag="rt", bufs=6)
                nc.vector.tensor_mul(m1[:], ac[:], cc2_sb[:, tl])
                nc.vector.tensor_mul(m2[:], bc[:], smp_sb[:, tl])
                nc.vector.tensor_mul(m3[:], bc[:], cc2_sb[:, tl])
                nc.vector.tensor_mul(m4[:], ac[:], smp_sb[:, tl])
                nc.vector.tensor_sub(d0[0:64, tl], m1[0:64, :], m2[0:64, :])
                nc.vector.tensor_sub(d1[64:128, tl], m1[64:128, :], m2[64:128, :])
                rb = work.tile([128, 512], bf16, name=f"rb{nm}", tag="rb", bufs=2)
                nc.vector.tensor_add(rb[:], m3[:], m4[:])
                nc.gpsimd.dma_start(d0[64:128, tl], rb[0:64, :])
                nc.gpsimd.dma_start(d1[0:64, tl], rb[64:128, :])

            def emit_B(rep, b):
                for tt in range(cfg.NT):
                    xb = xq.pop(0)
                    prefetch_x()
                    tl = slice(tt * 512, (tt + 1) * 512)
                    if rep == 0 and b == 0 and tt == 0:
                        # stream the rest of the startup state behind tile 0
                        for q in range(wstep, cfg.NCC, wstep):
                            nc.sync.dma_start(wv_sb[:, q:q + wstep, :],
                                              wv_d[:, q:q + wstep, :])
                            nc.sync.dma_start(wqk_sb[:, q:q + wstep, :],
                                              wqk_d[:, q:q + wstep, :])
                        nc.scalar.dma_start(cc2_sb[:, 512:cfg.T],
                                            cc2_d[:, 512:cfg.T])
                        nc.scalar.dma_start(smp_sb[:, 512:cfg.T],
                                            smp_d[:, 512:cfg.T])
                    # j order: v0, v1, qA, qB, kA, kB
                    pj = {}
                    for j in range(6):
                        p = psum(f"pj_{rep}_{b}_{tt}_{j}")
                        for ccs in range(cfg.NCC):
                            if j < 2:
                                w = wv_sb[:, ccs, j * 128:(j + 1) * 128]
                            else:
                                jc = j - 2
                                w = wqk_sb[:, ccs, jc * 128:(jc + 1) * 128]
                            nc.tensor.matmul(p[:], w, xb[:, ccs, :],
                                             start=(ccs == 0),
                                             stop=(ccs == cfg.NCC - 1))
                        pj[j] = p
                        if j == 1:
                            for hl in range(H_LOCAL):
                                vfm = work.tile([128, 512], bf16,
                                                name=f"vfm_{rep}_{b}_{tt}_{hl}",
                                                tag="vfm", bufs=3)
                                nc.scalar.activation(vfm[:], pj[hl][:], Copy)
                                nc.sync.dma_start_transpose(
                                    v_sb[(b, hl)][:, 4 * tt:4 * tt + 4, :],
                                    vfm[:])
                            if not (rep == 0 and b == 0 and tt == 0):
                                emit_pd()
                                if tt > 0:
                                    drain_D(1)
                        elif j == 3:
                            rope_group(pj[2], pj[3], qh_sb[(b, 0)], qh_sb[(b, 1)],
                                       tl, f"q{rep}_{b}_{tt}")
                            if tt > 0:
                                drain_D(1)
                        elif j == 5:
                            rope_group(pj[4], pj[5], kh_sb[(b, 0)], kh_sb[(b, 1)],
                                       tl, f"k{rep}_{b}_{tt}")
                            if tt > 0:
                                drain_D(1)

            # ---------- phase C: causal attention ----------
            def emit_C(rep, b):
                for tt in range(cfg.NT):
                    tl0 = tt * 512
                    n_sc = 4 * (tt + 1)
                    po = {h: po_fix[(tt % 2, h)] for h in range(H_LOCAL)}
                    acc0 = work.tile([128, 1024], bf16,
                                     name=f"a0_{rep}_{b}_{tt}", tag="acc0", bufs=2)
                    acc1 = None
                    if tt > 0:
                        acc1 = work.tile([128, 1024], bf16,
                                         name=f"a1_{rep}_{b}_{tt}", tag="acc1",
                                         bufs=2)
                    prs = {}

                    def emit_scores(sc):
                        # score matmuls + exp + mask for block sc (one ahead
                        # of the PV consumer so Act latency hides under PE)
                        off = max(0, (sc - 4 * tt) * 128)
                        sl = slice(sc * 128, (sc + 1) * 128)
                        tls = slice(tl0 + off, tl0 + 512)
                        ph = {}
                        for h in range(H_LOCAL):
                            p = psum(f"ph_{rep}_{b}_{tt}_{sc}_{h}")
                            nc.tensor.matmul(p[:, off:512],
                                             kh_sb[(b, h)][:, sl],
                                             qh_sb[(b, h)][:, tls],
                                             start=True, stop=True)
                            ph[h] = p
                        pr = work.tile([128, 1024], bf16,
                                       name=f"pr_{rep}_{b}_{tt}_{sc}",
                                       tag="pr", bufs=4)
                        for h in range(H_LOCAL):
                            nc.scalar.activation(pr[:, h * 512 + off:(h + 1) * 512],
                                                 ph[h][:, off:512], Exp,
                                                 scale=SCALE)
                        if sc >= 4 * tt:   # diagonal: mask the triangle block
                            for h in range(H_LOCAL):
                                pos = h * 512 + off
                                nc.gpsimd.tensor_mul(pr[:, pos:pos + 128],
                                                     pr[:, pos:pos + 128],
                                                     tri_sb[:])
                        prs[sc] = pr

                    emit_scores(0)
                    for sc in range(n_sc):
                        if sc + 1 < n_sc:
                            emit_scores(sc + 1)
                        off = max(0, (sc - 4 * tt) * 128)
                        wdt = 512 - off
                        pr = prs.pop(sc)
                        # denominator accumulation (bf16, two chains)
                        if sc == 0:
                            nc.vector.tensor_copy(acc0[:], pr[:])
                        elif sc == 1 and tt > 0:
                            nc.vector.tensor_copy(acc1[:], pr[:])
                        else:
                            tgt = acc0 if (tt == 0 or sc % 2 == 0) else acc1
                            if off == 0:
                                nc.vector.tensor_add(tgt[:], tgt[:], pr[:])
                            else:
                                for h in range(H_LOCAL):
                                    pos = h * 512 + off
                                    nc.gpsimd.tensor_add(tgt[:, pos:pos + wdt],
                                                         tgt[:, pos:pos + wdt],
                                                         pr[:, pos:pos + wdt])
                        for h in range(H_LOCAL):
                            nc.tensor.matmul(po[h][:, off:512],
                                             v_sb[(b, h)][:, sc, :],
                                             pr[:, h * 512 + off:(h + 1) * 512],
                                             start=(sc == 0),
                                             stop=(sc == n_sc - 1))
                        if sc == 1:
                            emit_pd()
                        if sc % 5 == 4:
                            drain_D(1)
                    # fold the two accumulator chains
                    if acc1 is not None:
                        accS = work.tile([128, 1024], bf16,
                                         name=f"as_{rep}_{b}_{tt}", tag="accS",
                                         bufs=2)
                        nc.vector.tensor_add(accS[:], acc0[:], acc1[:])
                    else:
                        accS = acc0
                    c_state[(b, tt)] = (po, accS)
                    pd_pending.append((b, tt))

            # ---------- main schedule ----------
            for rep in range(repeat):
                emit_B(rep, 0)
                emit_B(rep, 1)
                emit_C(rep, 0)
                emit_C(rep, 1)

            # tail: last tile's denominator + final projection phase
            emit_pd()
            drain_D(len(d_pending))

    nc.compile()
    return nc


# =====================================================================
# Host-side input prep / output gather
# =====================================================================

def _part_major(a2d, ncc):
    """[ncc*128, F] -> [128, ncc, F] with row r = chunk*128 + p."""
    F = a2d.shape[1]
    return np.ascontiguousarray(
        a2d.reshape(ncc, 128, F).transpose(1, 0, 2))


def make_trig(cfg: Cfg):
    pos = np.arange(cfg.T, dtype=np.float64)[None, :]        # [1,T]
    j = np.arange(64, dtype=np.float64)[:, None]             # [64,1]
    inv = ROPE_BASE ** (-2.0 * j / Dh)
    ang = pos * inv                                          # [64,T]
    sin = np.sin(ang).astype(np.float32)
    cos = np.cos(ang).astype(np.float32)
    cc2 = np.concatenate([cos, cos], axis=0).astype(BF16)    # [128,T]
    smp = np.concatenate([sin, -sin], axis=0).astype(BF16)
    return cc2, smp


def make_tri():
    p = np.arange(128)[:, None]
    jj = np.arange(128)[None, :]
    return (p <= jj).astype(BF16)


def make_in_maps(x, w_qkv, w_proj, cfg: Cfg = FULL, n_cores=N_CORES,
                 n_head=N_HEAD):
    x = np.asarray(x, np.float32)
    w_qkv = np.asarray(w_qkv, np.float32)
    w_proj = np.asarray(w_proj, np.float32)
    Cm = cfg.C

    xT = np.ascontiguousarray(x.reshape(cfg.BT, Cm).T)       # [C, BT]
    xt = _part_major(xT, cfg.NCC).astype(BF16)
    wp = _part_major(w_proj, cfg.GRP).astype(BF16)
    cc2, smp = make_trig(cfg)
    tri = make_tri()

    wq = w_qkv[:, 0:Cm]
    wk = w_qkv[:, Cm:2 * Cm]
    wv_all = w_qkv[:, 2 * Cm:3 * Cm]

    in_maps = []
    for c in range(n_cores):
        h0, h1 = 2 * c, 2 * c + 1
        q0 = wq[:, h0 * 128:(h0 + 1) * 128]
        q1 = wq[:, h1 * 128:(h1 + 1) * 128]
        k0 = wk[:, h0 * 128:(h0 + 1) * 128]
        k1 = wk[:, h1 * 128:(h1 + 1) * 128]
        qA = np.concatenate([q0[:, 0:64], q1[:, 64:128]], axis=1)
        qB = np.concatenate([q0[:, 64:128], q1[:, 0:64]], axis=1)
        kA = np.concatenate([k0[:, 0:64], k1[:, 64:128]], axis=1)
        kB = np.concatenate([k0[:, 64:128], k1[:, 0:64]], axis=1)
        wqk = _part_major(
            np.concatenate([qA, qB, kA, kB], axis=1), cfg.NCC).astype(BF16)
        wv = _part_major(
            np.concatenate([wv_all[:, h0 * 128:(h0 + 1) * 128],
                            wv_all[:, h1 * 128:(h1 + 1) * 128]], axis=1),
            cfg.NCC).astype(BF16)
        in_maps.append(dict(xt=xt, wqk=wqk, wv=wv, wp=wp,
                            cc2=cc2, smp=smp, tri=tri))
    return in_maps


def gather(outs, cfg: Cfg = FULL):
    """outs: per-core [B, H_LOCAL, 128, C] -> full [B, T, C]."""
    rows = np.concatenate(
        [o.reshape(cfg.B, H_LOCAL * 128, cfg.C) for o in outs], axis=1)
    return np.ascontiguousarray(rows.reshape(cfg.B, cfg.T, cfg.C))


# =====================================================================
# Public entry point
# =====================================================================

_NC_CACHE = {}


def get_nc(debug=False):
    key = ("full", debug)
    if key not in _NC_CACHE:
        _NC_CACHE[key] = build_nc(FULL, debug=debug)
    return _NC_CACHE[key]


def kernel(x, w_qkv, w_proj):
    from concourse.bass_utils import run_bass_kernel_spmd
    nc = get_nc()
    in_maps = make_in_maps(x, w_qkv, w_proj)
    res = run_bass_kernel_spmd(nc, in_maps, list(range(N_CORES)))
    return gather([res.results[c]["out"] for c in range(N_CORES)])
